# revision 26
# baseline (speedup 1.0000x reference)
"""Trainium2 Bass kernel for causal FFT convolution (nn_CausalConvolution).

y = irfft(rfft(bf16(x), 2T) * rfft(h, 2T))[..., :T],  x,h: (8, 64, 65536) fp32.

Identity: with z = bf16(x) + i*h,  y = Im(iFFT(FFT_2T(z)^2)) / 2.
N = 131072 decomposed radix (128, 128, 8); 512 channels sharded 64/core
across 8 NeuronCores (pure data parallelism).

v3 design vs the fp32 baseline:
  - bf16 data planes everywhere past stage 1 (PSUM accumulates fp32);
    bf16 stationaries enable FWL so LDWEIGHTS hides behind matmuls.
  - Inverse stages S3' and S2' run as transposed-output matmuls (the data
    tile is the stationary operand), which eliminates the TR3/TR4
    PE-transposes and their PSUM-evacuation passes entirely.
  - The zero-padded first stage is row-tiled: two concurrent K=64 matmuls.
  - PSUM evacuations are fused with the twiddle / square / cast work and
    spread across ACT, DVE, and GPSIMD.
  - Final output stored in (q, bp, a)-permuted order; host unpermutes.

Layouts per 2-channel block (t = u*1024 + m, m = 8a + b; c = chi*16 + c16):
  B1  [k1, (ch, b, a)]            B5  [(c16*8+e), (ch, chi, d)]
  B2  [a,  (b, ch, k1)]           B7  [d,  (ch, bp, chi, c16)]
  B3  [d,  (ch, chi, c16, b)]     B9  [c,  (ch, bp, a)]
  B4  [(c16*8+b), (ch, chi, d)]   y   [u, q*512 + bpq*128 + a]

Self-contained: shapes/sharding hardcoded; tables computed with numpy here.
"""
import numpy as np
import ml_dtypes
from contextlib import ExitStack

import concourse.bass as bass
import concourse.bacc as bacc
import concourse.tile as tile
import concourse.mybir as mybir
from concourse.bass_utils import run_bass_kernel_spmd

F32 = mybir.dt.float32
F32R = mybir.dt.float32r
BF16 = mybir.dt.bfloat16
MUL = mybir.AluOpType.mult
ADD = mybir.AluOpType.add
SUB = mybir.AluOpType.subtract

Bsz, Csz, T = 8, 64, 65536
NFFT = 2 * T
NCORES = 8
CPC = (Bsz * Csz) // NCORES          # 64 channels per core
NBLK = CPC // 2                      # 2 channels per block

_Wc = lambda M, E: np.exp(-2j * np.pi * E / M)


def _gen_tables():
    F128 = _Wc(128, np.outer(np.arange(128), np.arange(128)))
    F8 = _Wc(8, np.outer(np.arange(8), np.arange(8)))
    W1024bd = _Wc(1024, np.outer(np.arange(8), np.arange(128)))     # [b, d]
    TW1_cab = _Wc(NFFT, (8 * np.arange(128)[None, :, None]
                         + np.arange(8)[None, None, :])
                  * np.arange(128)[:, None, None])                  # [c, a, b]

    f32 = lambda v: np.ascontiguousarray(v, dtype=np.float32)
    t = {}
    # S1 stationaries, rows duplicated for 2-way row tiling: [128, 256]
    s1re, s1im = F128[:64].real, F128[:64].imag
    t["s1bf"] = f32(np.concatenate([np.tile(s1re, (2, 1)),
                                    np.tile(s1im, (2, 1))], axis=1))
    t["s1fr"] = f32(np.concatenate([np.tile(-s1im, (2, 1)),
                                    np.tile(s1re, (2, 1))], axis=1))

    mats, idx = [], {}

    def put(name, m):
        idx[name] = len(mats)
        mats.append(f32(m))

    S2 = F128[None, :, :] * W1024bd[:, None, :]                     # [b, a, d]
    for b in range(8):
        put(f"s2_re{b}", S2[b].real)
        put(f"s2_im{b}", S2[b].imag)
        put(f"s2_imn{b}", -S2[b].imag)
    S3 = np.zeros((128, 128), np.complex128)
    for b in range(8):
        for e in range(8):
            for c16 in range(16):
                S3[c16 * 8 + b, c16 * 8 + e] = F8[b, e]
    put("s3_re", S3.real)
    put("s3_im", S3.imag)
    put("s3_imn", -S3.imag)
    # S3' as TOM rhs: psum[d, (c16,bp)] = sum_p B5[p, d] * S3p[p, (c16,bp)]
    S3p = np.zeros((128, 128), np.complex128)
    for e in range(8):
        for bp in range(8):
            for c16 in range(16):
                S3p[c16 * 8 + e, c16 * 8 + bp] = np.conj(F8[bp, e])
    put("s3p_re", S3p.real)          # data_re -> psum_re
    put("s3p_im", S3p.imag)          # data_re -> psum_im
    put("s3p_imn2", -2 * S3p.imag)   # data_im -> psum_re (x2: Sim halved)
    put("s3p_re2", 2 * S3p.real)     # data_im -> psum_im
    # S2' as TOM rhs: psum[c, a] = sum_d B7[d, c] * S2p[d, a]
    S2p = np.conj(S2).transpose(0, 2, 1)                            # [bp, d, a]
    for b in range(8):
        put(f"s2p_re{b}", S2p[b].real)
        put(f"s2p_im{b}", S2p[b].imag)
        put(f"s2p_imn{b}", -S2p[b].imag)
    put("ident", np.eye(128))
    t["st"] = np.stack(mats)                                        # bf16 on hw
    t["st_idx"] = idx

    # S1' [c, u] scaled 1/(2N), full M=128 (rows 64-127 unused): [128, 256]
    S1p = np.conj(F128).T / (2.0 * NFFT)
    t["s1p"] = f32(np.concatenate([S1p.imag, S1p.real], axis=1))

    # tw1 [k1, b*128 + a] = W_N^{k1*(8a+b)}: [128, 2048] = [re | im]
    m_ba = TW1_cab.transpose(0, 2, 1).reshape(128, 1024)
    t["tw1"] = f32(np.concatenate([m_ba.real, m_ba.imag], axis=1))
    # tw2p [c, bp*128 + a] = conj(W_N^{c*(8a+bp)}): [128, 2048] = [re | im]
    m2 = np.conj(TW1_cab).transpose(0, 2, 1).reshape(128, 1024)
    t["tw2p"] = f32(np.concatenate([m2.real, m2.imag], axis=1))
    return t


def _build(n_blocks=NBLK, debug=False, stop=None):
    tabs = _gen_tables()
    nc = bacc.Bacc("TRN2", target_bir_lowering=False, debug=False)

    x_d = nc.dram_tensor("x_in", [CPC, 64, 1024], BF16, kind="ExternalInput").ap()
    h_d = nc.dram_tensor("h_in", [CPC, 64, 1024], F32R, kind="ExternalInput").ap()
    nst = tabs["st"].shape[0]
    st_d = nc.dram_tensor("st_in", [nst, 128, 128], BF16, kind="ExternalInput").ap()
    s1bf_d = nc.dram_tensor("s1bf_in", [128, 256], BF16, kind="ExternalInput").ap()
    s1fr_d = nc.dram_tensor("s1fr_in", [128, 256], F32R, kind="ExternalInput").ap()
    s1p_d = nc.dram_tensor("s1p_in", [128, 256], BF16, kind="ExternalInput").ap()
    tw1_d = nc.dram_tensor("tw1_in", [128, 2048], BF16, kind="ExternalInput").ap()
    tw2p_d = nc.dram_tensor("tw2p_in", [128, 2048], BF16, kind="ExternalInput").ap()
    y_d = nc.dram_tensor("y_out", [CPC, 64, 1024], F32, kind="ExternalOutput").ap()
    dbg_d = {}
    if debug:
        for nm in ["B1", "B2", "B3", "B4", "B5", "B7", "B9"]:
            for pl in ["re", "im"]:
                dbg_d[nm + pl] = nc.dram_tensor(
                    f"dbg_{nm}{pl}", [128, 2048], BF16, kind="ExternalOutput").ap()

    with tile.TileContext(nc) as tc, ExitStack() as ctx:
        const = ctx.enter_context(tc.tile_pool(name="const", bufs=1))
        data = ctx.enter_context(tc.tile_pool(name="io", bufs=3))
        sbA = ctx.enter_context(tc.tile_pool(name="sbA", bufs=3))
        sbB = ctx.enter_context(tc.tile_pool(name="sbB", bufs=3))
        sbC = ctx.enter_context(tc.tile_pool(name="sbC", bufs=3))
        tmp = ctx.enter_context(tc.tile_pool(name="tmp", bufs=4))
        psum = ctx.enter_context(tc.tile_pool(name="psum", bufs=3, space="PSUM"))
        psumT = ctx.enter_context(tc.tile_pool(name="psumT", bufs=2, space="PSUM"))

        # ---- load constant tables once ----
        st = const.tile([128, nst * 128], BF16, tag="st")
        nc.sync.dma_start(
            st[:].rearrange("p (n c) -> p n c", n=nst),
            st_d.rearrange("n p c -> p n c"))
        s1bf = const.tile([128, 256], BF16, tag="s1bf")
        nc.sync.dma_start(s1bf[:], s1bf_d)
        s1fr = const.tile([128, 256], F32R, tag="s1fr")
        nc.sync.dma_start(s1fr[:], s1fr_d)
        s1p = const.tile([128, 256], BF16, tag="s1p")
        nc.sync.dma_start(s1p[:], s1p_d)
        tw1 = const.tile([128, 2048], BF16, tag="tw1")
        nc.sync.dma_start(tw1[:], tw1_d)
        tw2p = const.tile([128, 2048], BF16, tag="tw2p")
        nc.sync.dma_start(tw2p[:], tw2p_d)

        tc.strict_bb_all_engine_barrier()

        sidx = tabs["st_idx"]
        M = lambda name: st[:, sidx[name] * 128:(sidx[name] + 1) * 128]
        ident = M("ident")
        tw1_re, tw1_im = tw1[:, 0:1024], tw1[:, 1024:2048]
        tw2p_re, tw2p_im = tw2p[:, 0:1024], tw2p[:, 1024:2048]

        def cmm(pre, pim, mrr, mir, mri, mii, rre, rim):
            """pre = mrr.T@rre + mir.T@rim ; pim = mri.T@rre + mii.T@rim"""
            nc.tensor.matmul(pre, mrr, rre, start=True, stop=False)
            nc.tensor.matmul(pre, mir, rim, start=False, stop=True)
            nc.tensor.matmul(pim, mri, rre, start=True, stop=False)
            nc.tensor.matmul(pim, mii, rim, start=False, stop=True)

        def cmm_tom(pre, pim, dre, dim, mrr, mri, mirn, mii):
            """Transposed-output: pre = dre.T@mrr + dim.T@mirn;
            pim = dre.T@mri + dim.T@mii. Grouped by stationary (2 LDW)."""
            nc.tensor.matmul(pre, dre, mrr, start=True, stop=False)
            nc.tensor.matmul(pim, dre, mri, start=True, stop=False)
            nc.tensor.matmul(pre, dim, mirn, start=False, stop=True)
            nc.tensor.matmul(pim, dim, mii, start=False, stop=True)

        def pair():
            pr = psum.tile([128, 512], F32, tag="pr")
            pi = psum.tile([128, 512], F32, tag="pi")
            return pr, pi

        def dbg_tap(name, tre, tim):
            if debug:
                nc.sync.dma_start(dbg_d[name + "re"][:], tre[:])
                nc.sync.dma_start(dbg_d[name + "im"][:], tim[:])

        for blk in range(n_blocks):
            ch0 = 2 * blk
            # ---- load: xq bf16 row-packed [h*64+u, ch*512+mm], him f32r ----
            xq = data.tile([128, 1024], BF16, tag="xq")
            him = data.tile([128, 1024], F32R, tag="him")
            for ch in range(2):
                for h in range(2):
                    rs = slice(h * 64, (h + 1) * 64)
                    cs = slice(ch * 512, (ch + 1) * 512)
                    nc.sync.dma_start(
                        xq[rs, cs], x_d[ch0 + ch][:, h * 512:(h + 1) * 512])
                    nc.sync.dma_start(
                        him[rs, cs], h_d[ch0 + ch][:, h * 512:(h + 1) * 512])

            # ---- S1 (row-tiled K=64 pairs) + EV1a cast-permute ----
            b1t_re = sbA.tile([128, 2048], BF16, tag="A_re")
            b1t_im = sbA.tile([128, 2048], BF16, tag="A_im")
            v1re = b1t_re[:].rearrange("p (ch b a) -> p ch b a", ch=2, b=8, a=128)
            v1im = b1t_im[:].rearrange("p (ch b a) -> p ch b a", ch=2, b=8, a=128)
            for ch in range(2):
                for h in range(2):
                    rs = slice(h * 64, (h + 1) * 64)
                    cs = slice(ch * 512, (ch + 1) * 512)
                    pr, pi = pair()
                    cmm(pr[:], pi[:],
                        s1bf[rs, 0:128], s1fr[rs, 0:128],
                        s1bf[rs, 128:256], s1fr[rs, 128:256],
                        xq[rs, cs], him[rs, cs])
                    asl = slice(h * 64, (h + 1) * 64)
                    nc.scalar.copy(
                        v1re[:, ch, :, asl].rearrange("p b a -> p a b"),
                        pr[:].rearrange("p (a b) -> p a b", a=64, b=8))
                    nc.vector.tensor_copy(
                        v1im[:, ch, :, asl].rearrange("p b a -> p a b"),
                        pi[:].rearrange("p (a b) -> p a b", a=64, b=8))

            # ---- EV1b twiddle (bf16) per ch ----
            b1_re = sbB.tile([128, 2048], BF16, tag="B_re")
            b1_im = sbB.tile([128, 2048], BF16, tag="B_im")
            for ch in range(2):
                cs = slice(ch * 1024, (ch + 1) * 1024)
                t1 = tmp.tile([128, 1024], BF16, tag="t1")
                t2 = tmp.tile([128, 1024], BF16, tag="t2")
                t3 = tmp.tile([128, 1024], BF16, tag="t3")
                t4 = tmp.tile([128, 1024], BF16, tag="t4")
                nc.vector.tensor_tensor(t1[:], b1t_re[:, cs], tw1_re, MUL)
                nc.gpsimd.tensor_tensor(t2[:], b1t_im[:, cs], tw1_im, MUL)
                nc.vector.tensor_tensor(t3[:], b1t_re[:, cs], tw1_im, MUL)
                nc.gpsimd.tensor_tensor(t4[:], b1t_im[:, cs], tw1_re, MUL)
                nc.vector.tensor_tensor(b1_re[:, cs], t1[:], t2[:], SUB)
                nc.vector.tensor_tensor(b1_im[:, cs], t3[:], t4[:], ADD)
            dbg_tap("B1", b1_re, b1_im)
            if stop == "B1":
                continue

            # ---- TR1 (PE) + EV2: B2 [a, (b, ch, k1)] ----
            b2_re = sbC.tile([128, 2048], BF16, tag="C_re")
            b2_im = sbC.tile([128, 2048], BF16, tag="C_im")
            v2re = b2_re[:].rearrange("p (b ch k) -> p b ch k", b=8, ch=2, k=128)
            v2im = b2_im[:].rearrange("p (b ch k) -> p b ch k", b=8, ch=2, k=128)
            for ch in range(2):
                for pl in range(2):
                    src, vdst, eng = ((b1_re, v2re, nc.scalar) if pl == 0
                                      else (b1_im, v2im, nc.vector))
                    pt = psumT.tile([128, 1024], BF16, tag="pt")
                    for b in range(8):
                        ss = slice(ch * 1024 + b * 128, ch * 1024 + b * 128 + 128)
                        nc.tensor.transpose(
                            pt[:, b * 128:(b + 1) * 128], src[:, ss], ident)
                    dst = vdst[:, :, ch, :]
                    if eng is nc.scalar:
                        eng.copy(dst, pt[:].rearrange("p (b k) -> p b k", b=8))
                    else:
                        eng.tensor_copy(
                            dst, pt[:].rearrange("p (b k) -> p b k", b=8))
            dbg_tap("B2", b2_re, b2_im)
            if stop == "B2":
                continue

            # ---- S2 (contract a, per b) + EV3 cast-permute ----
            b3_re = sbA.tile([128, 2048], BF16, tag="A_re")
            b3_im = sbA.tile([128, 2048], BF16, tag="A_im")
            v3re = b3_re[:].rearrange("p (ch chi c16 b) -> p ch chi c16 b",
                                      ch=2, chi=8, c16=16, b=8)
            v3im = b3_im[:].rearrange("p (ch chi c16 b) -> p ch chi c16 b",
                                      ch=2, chi=8, c16=16, b=8)
            for hb in range(4):
                pr, pi = pair()
                for j in range(2):
                    b = hb * 2 + j
                    s = slice(j * 256, (j + 1) * 256)
                    rs = slice(b * 256, (b + 1) * 256)
                    cmm(pr[:, s], pi[:, s],
                        M(f"s2_re{b}"), M(f"s2_imn{b}"),
                        M(f"s2_im{b}"), M(f"s2_re{b}"),
                        b2_re[:, rs], b2_im[:, rs])
                jsl = slice(hb * 2, hb * 2 + 2)
                for ps, ov, eng in ((pr, v3re, nc.scalar), (pi, v3im, nc.vector)):
                    dst = ov[:, :, :, :, jsl].rearrange(
                        "p ch chi c16 j -> p j ch chi c16")
                    srcv = ps[:].rearrange(
                        "p (j ch chi c16) -> p j ch chi c16",
                        j=2, ch=2, chi=8, c16=16)
                    if eng is nc.scalar:
                        eng.copy(dst, srcv)
                    else:
                        eng.tensor_copy(dst, srcv)
            dbg_tap("B3", b3_re, b3_im)
            if stop == "B3":
                continue

            # ---- TR2 (PE) + EV4: B4 [(c16*8+b), (ch, chi, d)] ----
            b4_re = sbB.tile([128, 2048], BF16, tag="B_re")
            b4_im = sbB.tile([128, 2048], BF16, tag="B_im")
            for ch in range(2):
                for pl in range(2):
                    src, dstt, eng = ((b3_re, b4_re, nc.scalar) if pl == 0
                                      else (b3_im, b4_im, nc.vector))
                    pt = psumT.tile([128, 1024], BF16, tag="pt")
                    for chi in range(8):
                        ss = slice(ch * 1024 + chi * 128,
                                   ch * 1024 + chi * 128 + 128)
                        nc.tensor.transpose(
                            pt[:, chi * 128:(chi + 1) * 128], src[:, ss], ident)
                    dst = dstt[:, ch * 1024:(ch + 1) * 1024]
                    if eng is nc.scalar:
                        eng.copy(dst, pt[:])
                    else:
                        eng.tensor_copy(dst, pt[:])
            dbg_tap("B4", b4_re, b4_im)
            if stop == "B4":
                continue

            # ---- S3 + EV5 square: B5 = (Sre, Sim/2) ----
            b5_re = sbA.tile([128, 2048], BF16, tag="A_re")
            b5_im = sbA.tile([128, 2048], BF16, tag="A_im")
            for ck in range(4):
                cs = slice(ck * 512, (ck + 1) * 512)
                pr, pi = pair()
                cmm(pr[:], pi[:], M("s3_re"), M("s3_imn"), M("s3_im"), M("s3_re"),
                    b4_re[:, cs], b4_im[:, cs])
                tP = tmp.tile([128, 512], BF16, tag="s1t")
                tQ = tmp.tile([128, 512], BF16, tag="s2t")
                t1 = tmp.tile([128, 512], BF16, tag="s3t")
                t2 = tmp.tile([128, 512], BF16, tag="s4t")
                nc.scalar.copy(tP[:], pr[:])
                nc.scalar.copy(tQ[:], pi[:])
                nc.vector.tensor_tensor(t1[:], tP[:], tP[:], MUL)
                nc.gpsimd.tensor_tensor(t2[:], tQ[:], tQ[:], MUL)
                nc.vector.tensor_tensor(b5_re[:, cs], t1[:], t2[:], SUB)
                nc.vector.tensor_tensor(b5_im[:, cs], tP[:], tQ[:], MUL)
            dbg_tap("B5", b5_re, b5_im)
            if stop == "B5":
                continue

            # ---- S3' (TOM) + EV6: B7 [d, (ch, bp, chi, c16)] ----
            b7_re = sbB.tile([128, 2048], BF16, tag="B_re")
            b7_im = sbB.tile([128, 2048], BF16, tag="B_im")
            v7re = b7_re[:].rearrange("p (ch bp chi c16) -> p ch bp chi c16",
                                      ch=2, bp=8, chi=8, c16=16)
            v7im = b7_im[:].rearrange("p (ch bp chi c16) -> p ch bp chi c16",
                                      ch=2, bp=8, chi=8, c16=16)
            for ch in range(2):
                for hc in range(2):
                    pr, pi = pair()
                    for g in range(4):
                        chi = hc * 4 + g
                        ss = slice(ch * 1024 + chi * 128,
                                   ch * 1024 + chi * 128 + 128)
                        s = slice(g * 128, (g + 1) * 128)
                        cmm_tom(pr[:, s], pi[:, s],
                                b5_re[:, ss], b5_im[:, ss],
                                M("s3p_re"), M("s3p_im"),
                                M("s3p_imn2"), M("s3p_re2"))
                    # psum cols (g, c16, bp) -> B7 [d, ch, bp, chi=hc*4+g, c16]
                    for ps, ov, eng in ((pr, v7re, nc.scalar),
                                        (pi, v7im, nc.vector)):
                        dst = ov[:, ch, :, hc * 4:(hc + 1) * 4, :].rearrange(
                            "p bp g c16 -> p g c16 bp")
                        srcv = ps[:].rearrange("p (g c16 bp) -> p g c16 bp",
                                               g=4, c16=16, bp=8)
                        if eng is nc.scalar:
                            eng.copy(dst, srcv)
                        else:
                            eng.tensor_copy(dst, srcv)
            dbg_tap("B7", b7_re, b7_im)
            if stop == "B7":
                continue

            # ---- S2' (TOM) + EV8 cast+twiddle': B9 [c, (ch, bp, a)] ----
            b9_re = sbC.tile([128, 2048], BF16, tag="C_re")
            b9_im = sbC.tile([128, 2048], BF16, tag="C_im")
            for ch in range(2):
                for hp in range(2):
                    pr, pi = pair()
                    for g in range(4):
                        bp = hp * 4 + g
                        ss = slice(ch * 1024 + bp * 128,
                                   ch * 1024 + bp * 128 + 128)
                        s = slice(g * 128, (g + 1) * 128)
                        cmm_tom(pr[:, s], pi[:, s],
                                b7_re[:, ss], b7_im[:, ss],
                                M(f"s2p_re{bp}"), M(f"s2p_im{bp}"),
                                M(f"s2p_imn{bp}"), M(f"s2p_re{bp}"))
                    # psum cols (g, a); tw2p cols bp*128+a
                    cs = slice(ch * 1024 + hp * 512, ch * 1024 + (hp + 1) * 512)
                    ts_ = slice(hp * 512, (hp + 1) * 512)
                    tP = tmp.tile([128, 512], BF16, tag="s1t")
                    tQ = tmp.tile([128, 512], BF16, tag="s2t")
                    t1 = tmp.tile([128, 512], BF16, tag="s3t")
                    t2 = tmp.tile([128, 512], BF16, tag="s4t")
                    nc.scalar.copy(tP[:], pr[:])
                    nc.scalar.copy(tQ[:], pi[:])
                    nc.vector.tensor_tensor(t1[:], tP[:], tw2p_re[:, ts_], MUL)
                    nc.gpsimd.tensor_tensor(t2[:], tQ[:], tw2p_im[:, ts_], MUL)
                    nc.vector.tensor_tensor(b9_re[:, cs], t1[:], t2[:], SUB)
                    nc.vector.tensor_tensor(t1[:], tP[:], tw2p_im[:, ts_], MUL)
                    nc.gpsimd.tensor_tensor(t2[:], tQ[:], tw2p_re[:, ts_], MUL)
                    nc.vector.tensor_tensor(b9_im[:, cs], t1[:], t2[:], ADD)
            dbg_tap("B9", b9_re, b9_im)
            if stop == "B9":
                continue

            # ---- S1' (M=128) + EV10 + store ----
            # y_out col = q*512 + bpq*128 + a; host unpermutes to m' = 8a+4q+bpq.
            for ch in range(2):
                yt = data.tile([64, 1024], F32, tag="yt")
                for q in range(2):
                    rs = slice(ch * 1024 + q * 512, ch * 1024 + (q + 1) * 512)
                    p10 = psum.tile([128, 512], F32, tag="pr")
                    nc.tensor.matmul(p10[:], s1p[:, 0:128], b9_re[:, rs],
                                     start=True, stop=False)
                    nc.tensor.matmul(p10[:], s1p[:, 128:256], b9_im[:, rs],
                                     start=False, stop=True)
                    if ch == 0:
                        nc.scalar.copy(yt[:, q * 512:(q + 1) * 512], p10[0:64, :])
                    else:
                        nc.vector.tensor_copy(yt[:, q * 512:(q + 1) * 512],
                                              p10[0:64, :])
                nc.sync.dma_start(y_d[ch0 + ch], yt[:])

    nc.compile()
    return nc, tabs


_CACHE = {}


def _get(n_blocks=NBLK, debug=False, stop=None):
    key = (n_blocks, debug, stop)
    if key not in _CACHE:
        _CACHE[key] = _build(n_blocks, debug, stop)
    return _CACHE[key]


def _in_maps(x, h, tabs):
    xf = np.ascontiguousarray(x, np.float32).reshape(Bsz * Csz, 65536)
    hf = np.ascontiguousarray(h, np.float32).reshape(Bsz * Csz, 65536)
    b16 = lambda v: v.astype(ml_dtypes.bfloat16)
    maps = []
    for i in range(NCORES):
        sl = slice(i * CPC, (i + 1) * CPC)
        maps.append({
            "x_in": b16(xf[sl].reshape(CPC, 64, 1024)),
            "h_in": hf[sl].reshape(CPC, 64, 1024),
            "st_in": b16(tabs["st"]),
            "s1bf_in": b16(tabs["s1bf"]),
            "s1fr_in": tabs["s1fr"],
            "s1p_in": b16(tabs["s1p"]),
            "tw1_in": b16(tabs["tw1"]),
            "tw2p_in": b16(tabs["tw2p"]),
        })
    return maps


def _unpermute_y(y_out):
    """y_out [CPC, 64, 1024], col = q*512 + bpq*128 + a -> m' = 8a+4q+bpq."""
    v = np.asarray(y_out, np.float32).reshape(CPC * 64, 2, 4, 128)
    return np.ascontiguousarray(v.transpose(0, 3, 1, 2)).reshape(CPC, 65536)


def kernel(x, h):
    nc, tabs = _get()
    maps = _in_maps(x, h, tabs)
    res = run_bass_kernel_spmd(nc, maps, core_ids=list(range(NCORES)))
    y = np.concatenate([_unpermute_y(r["y_out"]) for r in res.results])
    return y.reshape(Bsz, Csz, T).astype(np.float32)
